# revision 25
# baseline (speedup 1.0000x reference)
"""Trainium2 Bass kernel for the blocked sparse-attention nn.Module.

Sharding: 8 cores = 4 batches x 2 T-halves (no collectives). Core c handles
batch b=c//2, half=c%2: original heads [8*half,8*half+8) == view-groups
[4*half,4*half+4) == output rows [half*2048,(half+1)*2048).

Per-core pipeline, per view-group gl in 0..4:
  1. Q^T/K^T projection in transposed layout [128ch, 4096t] (float32r
     matmuls, weights host-permuted to [A-even,A-odd,B-even,B-odd] rows).
  2. RoPE: QT = ctab*raw - Pswap32@(stab*raw) (constant-swap PE matmul);
     B-half extracted to partition base 0 for the mixed causal combos.
  3. V projection in natural layout [512t, 1024c] from a separate xTv copy.
  4. Per view-head h': S^T chunks (keys on partitions) -> exp(scale=1/8) on
     ACT -> UT-mask multiplies -> P@V with concurrent ones-row matmul
     (tile_position=(0,64)) accumulating rowsums -> reciprocal + rank-1
     PE broadcast -> normalized o^T tiles.
  5. out = o @ Wo from o^T tiles; qb/kb via gpsimd weighted reductions
     (host folds the two partition halves); vb via thin PE matmuls.
"""

import os

os.environ.setdefault("MYCRO_LOCAL_CACHE", "1")

import numpy as np

B, T, C = 4, 4096, 1024
H, G = 16, 8
HS = 64
NCORES = 8

# channel permutation within a 64-ch head: rows = [evens, odds]
_PI = np.concatenate([np.arange(0, 64, 2), np.arange(1, 64, 2)])
_ROW_OF_CH = np.argsort(_PI)  # row index of original channel e

_CACHE = {}


# --------------------------------------------------------------------------
# host-side prep / fold
# --------------------------------------------------------------------------
def _host_constants(Wqkv, Wo, q_proj, k_proj, v_proj, fc, fs):
    cosT = np.ascontiguousarray(fc.T).astype(np.float32)   # [32, T]
    sinT = np.ascontiguousarray(fs.T).astype(np.float32)
    ctab = np.concatenate([cosT, cosT, cosT, cosT], 0)     # [128, T]
    stab = np.concatenate([-sinT, sinT, -sinT, sinT], 0)

    swap32 = np.concatenate([np.arange(32, 64), np.arange(0, 32),
                             np.arange(96, 128), np.arange(64, 96)])
    pswap32 = np.eye(128, dtype=np.float32)[swap32].T.copy()
    p64h = np.zeros((128, 64), np.float32)
    p64h[np.arange(64, 128), np.arange(64)] = 1.0
    maskut = np.triu(np.ones((128, 128), np.float32))

    q1 = np.asarray(q_proj).reshape(-1)
    k1 = np.asarray(k_proj).reshape(-1)
    v1 = np.asarray(v_proj).reshape(-1)
    qprojw = np.concatenate([np.repeat(q1[0:256][None], 64, 0),
                             np.repeat(q1[256:512][None], 64, 0)], 0)
    kprojw = np.concatenate([np.repeat(k1[0:256][None], 64, 0),
                             np.repeat(k1[256:512][None], 64, 0)], 0)
    vprojw = np.ascontiguousarray(v1.reshape(4, 128).T)

    return dict(ctab=ctab, stab=stab, pswap32=pswap32, p64h=p64h,
                maskut=maskut, qprojw=qprojw.astype(np.float32),
                kprojw=kprojw.astype(np.float32),
                vprojw=vprojw.astype(np.float32),
                wv=np.ascontiguousarray(Wqkv[:, 2048:3072]).astype(np.float32),
                wo=np.ascontiguousarray(Wo).astype(np.float32))


def _percore_inputs(x, Wqkv, consts, core):
    b, half = core // 2, core % 2
    xT = np.ascontiguousarray(x[b].T).astype(np.float32)
    xTv = np.ascontiguousarray(xT[:, half * 2048:(half + 1) * 2048])
    perm_cols = []
    for gl in range(4):
        hA = 8 * half + 2 * gl
        perm_cols.append(hA * 64 + _PI)
        perm_cols.append((hA + 1) * 64 + _PI)
    perm_cols = np.concatenate(perm_cols)
    m = dict(xT=xT, xTv=xTv,
             wq=np.ascontiguousarray(Wqkv[:, perm_cols]).astype(np.float32),
             wk=np.ascontiguousarray(Wqkv[:, 1024 + perm_cols]).astype(np.float32))
    m.update(consts)
    return m


def _fold_outputs(results):
    out = np.zeros((B, T, C), np.float32)
    qb = np.zeros((B, H, G - 1, 1, HS), np.float32)
    kb = np.zeros((B, H, G - 1, 1, HS), np.float32)
    vb = np.zeros((B, H, G - 1, 1, HS), np.float32)
    rows = _ROW_OF_CH
    for core in range(NCORES):
        b, half = core // 2, core % 2
        r = results[core]
        out[b, half * 2048:(half + 1) * 2048, :] = r["out_rows"]
        acc = r["qkb_acc"]
        vba = r["vb_acc"]
        for gl in range(4):
            g = 4 * half + gl
            if g >= G - 1:
                continue
            qa = acc[gl, :, 0:16]
            ka = acc[gl, :, 16:32]
            qb[b, :, g, 0, :] = (qa[rows, :] + qa[64 + rows, :]).T
            kb[b, :, g, 0, :] = (ka[rows, :] + ka[64 + rows, :]).T
            vb[b, :, g, 0, :] = vba[gl].reshape(16, 64)
    return (out, qb, kb, vb)


# --------------------------------------------------------------------------
# bass program
# --------------------------------------------------------------------------
def _build_program(n_groups=4, debug_ot=False):
    import concourse.bass as bass
    import concourse.tile as tile
    import concourse.mybir as mybir
    from concourse import bacc
    from contextlib import ExitStack

    F32 = mybir.dt.float32
    F32R = mybir.dt.float32r
    MUL = mybir.AluOpType.mult
    SUB = mybir.AluOpType.subtract
    BYP = mybir.AluOpType.bypass
    EXP = mybir.ActivationFunctionType.Exp

    nc = bacc.Bacc("TRN2", target_bir_lowering=False, debug=False,
                   enable_asserts=False, num_devices=NCORES)

    d_xT = nc.dram_tensor("xT", [1024, 4096], F32, kind="ExternalInput")
    d_xTv = nc.dram_tensor("xTv", [1024, 2048], F32, kind="ExternalInput")
    d_wq = nc.dram_tensor("wq", [1024, 512], F32, kind="ExternalInput")
    d_wk = nc.dram_tensor("wk", [1024, 512], F32, kind="ExternalInput")
    d_wv = nc.dram_tensor("wv", [1024, 1024], F32, kind="ExternalInput")
    d_wo = nc.dram_tensor("wo", [1024, 1024], F32, kind="ExternalInput")
    d_ctab = nc.dram_tensor("ctab", [128, 4096], F32, kind="ExternalInput")
    d_stab = nc.dram_tensor("stab", [128, 4096], F32, kind="ExternalInput")
    d_psw = nc.dram_tensor("pswap32", [128, 128], F32, kind="ExternalInput")
    d_p64 = nc.dram_tensor("p64h", [128, 64], F32, kind="ExternalInput")
    d_mask = nc.dram_tensor("maskut", [128, 128], F32, kind="ExternalInput")
    d_qpw = nc.dram_tensor("qprojw", [128, 256], F32, kind="ExternalInput")
    d_kpw = nc.dram_tensor("kprojw", [128, 256], F32, kind="ExternalInput")
    d_vpw = nc.dram_tensor("vprojw", [128, 4], F32, kind="ExternalInput")

    d_out = nc.dram_tensor("out_rows", [2048, 1024], F32, kind="ExternalOutput")
    d_qkb = nc.dram_tensor("qkb_acc", [4, 128, 32], F32, kind="ExternalOutput")
    d_vb = nc.dram_tensor("vb_acc", [4, 1024], F32, kind="ExternalOutput")
    d_dbg = (nc.dram_tensor("dbg_ot", [8, 128, 512], F32, kind="ExternalOutput")
             if debug_ot else None)
    d_dbgp = (nc.dram_tensor("dbg_p", [4, 128, 512], F32, kind="ExternalOutput")
              if debug_ot else None)
    d_dbgo = (nc.dram_tensor("dbg_o", [65, 512], F32, kind="ExternalOutput")
              if debug_ot else None)

    def r8(ap):  # [rows=c*128, N] dram slice -> [128, c, N]
        return ap.rearrange("(c p) j -> p c j", p=128)

    with tile.TileContext(nc) as tc, ExitStack() as ctx:
        ctx.enter_context(nc.allow_low_precision(
            reason="float32r operands for full-rate PE; fp22 == PE internal precision"))
        const = ctx.enter_context(tc.tile_pool(name="const", bufs=1))
        t_ctab = const.tile([128, 4096], F32, tag="ctab")
        t_stab = const.tile([128, 4096], F32, tag="stab")
        t_psw = const.tile([128, 128], F32, tag="psw")
        t_p64 = const.tile([128, 64], F32, tag="p64")
        t_mask = const.tile([128, 128], F32, tag="mask")
        t_qpw = const.tile([128, 256], F32, tag="qpw")
        t_kpw = const.tile([128, 256], F32, tag="kpw")
        t_vpw = const.tile([128, 4], F32, tag="vpw")
        t_ones = const.tile([128, 1], F32, tag="ones")
        t_ones64 = const.tile([1, 64], F32, tag="ones64")
        nc.sync.dma_start(t_ctab[:], d_ctab[:])
        nc.sync.dma_start(t_stab[:], d_stab[:])
        nc.sync.dma_start(t_psw[:], d_psw[:])
        nc.sync.dma_start(t_p64[:], d_p64[:])
        nc.sync.dma_start(t_mask[:], d_mask[:])
        nc.sync.dma_start(t_qpw[:], d_qpw[:])
        nc.sync.dma_start(t_kpw[:], d_kpw[:])
        nc.sync.dma_start(t_vpw[:], d_vpw[:])
        nc.vector.memset(t_ones[:].bitcast(F32), 1.0)
        nc.vector.memset(t_ones64[:].bitcast(F32), 1.0)

        p_xt = ctx.enter_context(tc.tile_pool(name="xt", bufs=2))
        p_xtv = ctx.enter_context(tc.tile_pool(name="xtv", bufs=1))
        p_wg = ctx.enter_context(tc.tile_pool(name="wg", bufs=1))
        p_wvo = ctx.enter_context(tc.tile_pool(name="wvo", bufs=2))
        p_big = ctx.enter_context(tc.tile_pool(name="big", bufs=1))
        p_ot = ctx.enter_context(tc.tile_pool(name="ot", bufs=8))
        p_pt = ctx.enter_context(tc.tile_pool(name="pt", bufs=2))
        p_sm = ctx.enter_context(tc.tile_pool(name="sm", bufs=2))
        p_out = ctx.enter_context(tc.tile_pool(name="oev", bufs=2))

        # PSUM: 8 banks total: proj(2) + psw(1) + psbh(1) + psbig(3) + psO(1)
        ps_a = ctx.enter_context(tc.tile_pool(name="psA", bufs=2, space="PSUM"))
        ps_b = ctx.enter_context(tc.tile_pool(name="psB", bufs=2, space="PSUM"))
        ps_c = ctx.enter_context(tc.tile_pool(name="psC", bufs=3, space="PSUM"))
        ps_d = ctx.enter_context(tc.tile_pool(name="psD", bufs=1, space="PSUM"))

        for gl in range(n_groups):
            # ---------------- Q^T/K^T projection + rope ----------------
            t_wqg = p_wg.tile([128, 8, 128], F32, tag="wqg")
            t_wkg = p_wg.tile([128, 8, 128], F32, tag="wkg")
            nc.sync.dma_start(t_wqg[:], r8(d_wq[:, gl * 128:(gl + 1) * 128]))
            nc.sync.dma_start(t_wkg[:], r8(d_wk[:, gl * 128:(gl + 1) * 128]))

            t_QT = p_big.tile([128, 4096], F32, tag="QT")
            t_KT = p_big.tile([128, 4096], F32, tag="KT")
            t_QB = p_big.tile([64, 4096], F32, tag="QB")
            t_V = p_big.tile([128, 4, 1024], F32, tag="V")

            for tt in range(8):
                t0 = tt * 512
                xts = []
                for cg in range(2):
                    xt = p_xt.tile([128, 4, 512], F32, tag="xt")
                    nc.sync.dma_start(
                        xt[:], r8(d_xT[cg * 512:(cg + 1) * 512, t0:t0 + 512]))
                    xts.append(xt)

                for which, wgt, tdst in (("q", t_wqg, t_QT), ("k", t_wkg, t_KT)):
                    ps = ps_a.tile([128, 512], F32, tag="proj")
                    for c8 in range(8):
                        nc.tensor.matmul(
                            ps[:], wgt[:, c8, :].bitcast(F32R),
                            xts[c8 // 4][:, c8 % 4, :].bitcast(F32R),
                            start=(c8 == 0), stop=(c8 == 7))
                    t_w = p_sm.tile([128, 512], F32, tag="ropew")
                    nc.vector.tensor_tensor(
                        t_w[:], ps[:], t_stab[:, t0:t0 + 512], MUL)
                    ps_sw = ps_b.tile([128, 512], F32, tag="psw")
                    nc.tensor.matmul(ps_sw[:], t_psw[:].bitcast(F32R),
                                     t_w[:].bitcast(F32R), start=True, stop=True)
                    nc.vector.tensor_tensor(
                        tdst[:, t0:t0 + 512], ps[:], t_ctab[:, t0:t0 + 512], MUL)
                    nc.vector.tensor_tensor(
                        tdst[:, t0:t0 + 512], tdst[:, t0:t0 + 512], ps_sw[:], SUB)
                    if which == "q":
                        ps_bh = ps_b.tile([64, 512], F32, tag="psbh")
                        nc.tensor.matmul(
                            ps_bh[:], t_p64[:].bitcast(F32R),
                            tdst[:, t0:t0 + 512].bitcast(F32R),
                            start=True, stop=True)
                        nc.scalar.copy(t_QB[:, t0:t0 + 512], ps_bh[:])

            # ---------------- V projection (local rows) ----------------
            for nh in range(2):
                wvn = []
                for cg in range(2):
                    t_wvn = p_wvo.tile([128, 4, 512], F32, tag="wvo")
                    nc.sync.dma_start(
                        t_wvn[:], r8(d_wv[cg * 512:(cg + 1) * 512,
                                         nh * 512:(nh + 1) * 512]))
                    wvn.append(t_wvn)
                for tch in range(4):
                    xtv = p_xtv.tile([128, 8, 128], F32, tag="xtv")
                    c0 = gl * 512 + tch * 128
                    nc.sync.dma_start(xtv[:], r8(d_xTv[:, c0:c0 + 128]))
                    psv = ps_c.tile([128, 512], F32, tag="psbig")
                    for c8 in range(8):
                        nc.tensor.matmul(
                            psv[:], xtv[:, c8, :].bitcast(F32R),
                            wvn[c8 // 4][:, c8 % 4, :].bitcast(F32R),
                            start=(c8 == 0), stop=(c8 == 7))
                    nc.scalar.copy(t_V[:, tch, nh * 512:(nh + 1) * 512],
                                   psv[:])

            # ---------------- attention ----------------
            ot_list = []
            t_qkb = p_sm.tile([128, 32], F32, tag="qkb")
            for hp in range(16):
                if hp % 2 == 0:
                    t_oj = p_ot.tile([128, 512], F32, tag="oT")
                    ot_list.append(t_oj)

                qA = t_QT[0:64, hp::16]
                qB = t_QT[64:128, hp::16]
                qBs = t_QB[0:64, hp::16]
                kA0 = t_KT[0:64, hp:2048:16]
                kA1 = t_KT[0:64, 2048 + hp::16]
                kB2 = t_KT[64:128, hp:2048:16]
                kB3 = t_KT[64:128, 2048 + hp::16]

                psS0 = ps_c.tile([128, 512], F32, tag="psbig")
                nc.tensor.matmul(psS0[:, 0:256], kA0.bitcast(F32R),
                                 qA.bitcast(F32R), start=True, stop=False)
                nc.tensor.matmul(psS0[:, 256:512], kA0.bitcast(F32R),
                                 qBs.bitcast(F32R), start=False, stop=True)
                t_P0 = p_pt.tile([128, 512], F32, tag="P0")
                nc.scalar.activation(t_P0[:], psS0[:], EXP, scale=0.125)
                nc.gpsimd.tensor_tensor(t_P0[:, 0:128], t_P0[:, 0:128],
                                        t_mask[:], MUL)

                psS1 = ps_c.tile([128, 512], F32, tag="psbig")
                nc.tensor.matmul(psS1[:, 0:256], kA1.bitcast(F32R),
                                 qA.bitcast(F32R), start=True, stop=False)
                nc.tensor.matmul(psS1[:, 256:512], kA1.bitcast(F32R),
                                 qBs.bitcast(F32R), start=False, stop=True)
                t_P1 = p_pt.tile([128, 512], F32, tag="P1")
                nc.scalar.activation(t_P1[:, 128:512], psS1[:, 128:512], EXP,
                                     scale=0.125)
                nc.gpsimd.tensor_tensor(t_P1[:, 128:256], t_P1[:, 128:256],
                                        t_mask[:], MUL)

                psS2 = ps_c.tile([128, 256], F32, tag="psbig")
                nc.tensor.matmul(psS2[:], kB2.bitcast(F32R), qB.bitcast(F32R),
                                 start=True, stop=True)
                t_P2 = p_pt.tile([128, 256], F32, tag="P2")
                nc.scalar.activation(t_P2[:], psS2[:], EXP, scale=0.125)
                nc.gpsimd.tensor_tensor(t_P2[:, 0:128], t_P2[:, 0:128],
                                        t_mask[:], MUL)

                psS3 = ps_c.tile([128, 256], F32, tag="psbig")
                nc.tensor.matmul(psS3[:], kB3.bitcast(F32R), qB.bitcast(F32R),
                                 start=True, stop=True)
                t_P3 = p_pt.tile([128, 256], F32, tag="P3")
                nc.gpsimd.memset(t_P3[:, 0:128].bitcast(F32), 0.0)
                nc.scalar.activation(t_P3[:, 128:256], psS3[:, 128:256], EXP,
                                     scale=0.125)
                nc.gpsimd.tensor_tensor(t_P3[:, 128:256], t_P3[:, 128:256],
                                        t_mask[:], MUL)

                # PV accumulation; chunk order 1,2,3,0a,0b keeps every
                # psum zero-region's start/stop well-formed.
                psO = ps_d.tile([96, 512], F32, tag="psO")
                vsl = lambda k: t_V[:, k, hp * 64:(hp + 1) * 64].bitcast(F32R)
                mm = nc.tensor.matmul
                ONE = t_ones[:].bitcast(F32R)
                P0 = t_P0[:].bitcast(F32R)
                # one open group per bank at a time (sim tracks regions
                # without partition base): V rows fully, then ones row.
                for dst, w, rhs, st, sp in (
                    (psO[0:64, 128:512], vsl(1), t_P1[:, 128:512], True, False),
                    (psO[0:64, 256:512], vsl(2), t_P2[:], False, False),
                    (psO[0:64, 256:512], vsl(3), t_P3[:, 0:256], False, False),
                    (psO[0:64, 0:128], vsl(0), t_P0[:, 0:128], False, False),
                    (psO[0:64, 128:512], vsl(0), t_P0[:, 128:512], False, True),
                    (psO[64:96, 128:512], ONE, t_P1[:, 128:512], True, False),
                    (psO[64:96, 256:512], ONE, t_P2[:], False, False),
                    (psO[64:96, 256:512], ONE, t_P3[:, 0:256], False, False),
                    (psO[64:96, 0:128], ONE, t_P0[:, 0:128], False, False),
                    (psO[64:96, 128:512], ONE, t_P0[:, 128:512], False, True),
                ):
                    tp = (0, 64) if dst.base_partition() == 64 else None
                    mm(dst, w, rhs.bitcast(F32R), start=st, stop=sp,
                       tile_position=tp)

                if debug_ot and gl == 0 and hp == 0:
                    nc.sync.dma_start(d_dbgp[0, :, 0:512], t_P0[:, 0:512])
                    nc.sync.dma_start(d_dbgp[1, :, 128:512], t_P1[:, 128:512])
                    nc.sync.dma_start(d_dbgp[2, :, 0:256], t_P2[:, 0:256])
                    nc.sync.dma_start(d_dbgp[3, :, 0:256], t_P3[:, 0:256])
                    t_dbgo = p_sm.tile([65, 512], F32, tag="dbgo")
                    nc.vector.tensor_copy(t_dbgo[:], psO[:])
                    nc.sync.dma_start(d_dbgo[:, :], t_dbgo[:])
                t_r = p_sm.tile([1, 512], F32, tag="recip")
                nc.vector.reciprocal(t_r[:], psO[64:65, :])
                psBr = ps_c.tile([64, 512], F32, tag="psbig")
                nc.tensor.matmul(psBr[:], t_ones64[:].bitcast(F32R),
                                 t_r[:].bitcast(F32R), start=True, stop=True)
                off = (hp % 2) * 64
                nc.scalar.copy(t_oj[off:off + 64, :], psO[0:64, :])
                nc.vector.tensor_tensor(t_oj[off:off + 64, :],
                                        t_oj[off:off + 64, :], psBr[:], MUL)

                t_dum = p_sm.tile([128, 256], F32, tag="dum")
                nc.vector.scalar_tensor_tensor(
                    out=t_dum[:], in0=t_QT[:, hp::16], scalar=1.0,
                    in1=t_qpw[:], op0=BYP, op1=MUL,
                    accum_out=t_qkb[:, hp:hp + 1])
                t_dum2 = p_sm.tile([128, 256], F32, tag="dum")
                nc.vector.scalar_tensor_tensor(
                    out=t_dum2[:], in0=t_KT[:, hp::16], scalar=1.0,
                    in1=t_kpw[:], op0=BYP, op1=MUL,
                    accum_out=t_qkb[:, 16 + hp:17 + hp])

            nc.sync.dma_start(d_qkb[gl], t_qkb[:])
            if debug_ot and gl == 0:
                for j in range(8):
                    nc.sync.dma_start(d_dbg[j], ot_list[j][:])

            # ---------------- vb ----------------
            for nh in range(2):
                psvb = ps_b.tile([1, 512], F32, tag="psbh")
                for k4 in range(4):
                    nc.tensor.matmul(
                        psvb[:], t_vpw[:, k4:k4 + 1].bitcast(F32R),
                        t_V[:, k4, nh * 512:(nh + 1) * 512].bitcast(F32R),
                        start=(k4 == 0), stop=(k4 == 3))
                t_vbs = p_sm.tile([1, 512], F32, tag="recip")
                nc.vector.tensor_copy(t_vbs[:], psvb[:])
                nc.sync.dma_start(d_vb[gl:gl + 1, nh * 512:(nh + 1) * 512],
                                  t_vbs[:])

            # ---------------- output projection ----------------
            for nh in range(2):
                won = []
                for cg in range(2):
                    t_won = p_wvo.tile([128, 4, 512], F32, tag="wvo")
                    nc.sync.dma_start(
                        t_won[:], r8(d_wo[cg * 512:(cg + 1) * 512,
                                         nh * 512:(nh + 1) * 512]))
                    won.append(t_won)
                for m4 in range(4):
                    psE = ps_a.tile([128, 512], F32, tag="proj")
                    for j in range(8):
                        nc.tensor.matmul(
                            psE[:],
                            ot_list[j][:, m4 * 128:(m4 + 1) * 128].bitcast(F32R),
                            won[j // 4][:, j % 4, :].bitcast(F32R),
                            start=(j == 0), stop=(j == 7))
                    t_oe = p_out.tile([128, 512], F32, tag="oev")
                    nc.scalar.copy(t_oe[:], psE[:])
                    nc.sync.dma_start(
                        d_out[gl * 512 + m4 * 128:gl * 512 + (m4 + 1) * 128,
                              nh * 512:(nh + 1) * 512], t_oe[:])

    nc.compile()
    return nc


# --------------------------------------------------------------------------
# entry point
# --------------------------------------------------------------------------
LAST_RESULTS = None


def kernel(x, Wqkv, Wo, q_proj, k_proj, v_proj, freqs_cos, freqs_sin):
    from concourse import bass_utils

    if "nc" not in _CACHE:
        _CACHE["nc"] = _build_program()
    nc = _CACHE["nc"]

    x = np.ascontiguousarray(np.asarray(x, np.float32))
    consts = _host_constants(np.asarray(Wqkv, np.float32),
                             np.asarray(Wo, np.float32),
                             np.asarray(q_proj, np.float32),
                             np.asarray(k_proj, np.float32),
                             np.asarray(v_proj, np.float32),
                             np.asarray(freqs_cos, np.float32),
                             np.asarray(freqs_sin, np.float32))
    in_maps = [_percore_inputs(x, np.asarray(Wqkv, np.float32), consts, c)
               for c in range(NCORES)]

    res = bass_utils.run_bass_kernel_spmd(nc, in_maps,
                                          core_ids=list(range(NCORES)))
    global LAST_RESULTS
    LAST_RESULTS = res
    return _fold_outputs(res.results)
